# revision 19
# baseline (speedup 1.0000x reference)
"""Bass/Trainium2 kernel for BiDAF-style bidirectional attention.

Problem shapes (hardcoded per the task contract):
  context  (32, 1024, 512) f32
  question (32,  128, 512) f32
  ws       (1536,)         f32   = [ws_q | ws_c | ws_m]
Returns (c2q (32,1024,512), q2c (32,1,512)) matching reference.py.

Sharding: data-parallel over batch. 8 cores x 4 batches each; one SPMD
Bass module processes 4 batches per core.

Per-core per-batch dataflow (C=1024, Q=128, E=512):
  sim[c,q] = cdot[c] + qdot[q] + sum_e ctx[c,e]*ws_m[e]*q[q,e]
  - load ctx natural [c,e]; PE-transpose into ctxT [e,c] (4 e-chunks x 1024)
  - qs = q * ws_m (bcast); qsT via PE transpose; qdot via DVE fused
    mul+reduce; cdot row via 8 matmuls with 1-col ws_c weights over ctxT
  - simT[q,c] = qsT.T @ ctxT (+ cdot via an extra K=1 accumulating matmul)
  - expsimT = exp(simT + qdot) fused in the PSUM->SBUF ACT evacuation
    (no max subtraction: |sim| <~ 15 for this data, exp stays in fp32 range;
     softmax is shift-invariant so the result matches the reference)
  - PE-transpose expsimT back to [c,q]; DVE reduce gives s[c] = sum_q exp
    and w[c] = max_q exp = exp(max_q sim)  (the q2c softmax numerator!)
  - c2q[c,:] = (expsimT cols).T @ question, scaled by 1/s[c] during the
    ACT Copy evacuation (per-partition scale)
  - q2c = sum_c w[c]*ctx[c,:] / sum_c w[c] via 1-col-weight matmuls
"""

import os
import sys

for _p in ("/opt/trn_rl_repo",):
    if _p not in sys.path:
        sys.path.insert(0, _p)

import numpy as np

import concourse.bass as bass
import concourse.tile as tile
from concourse import mybir
from concourse.bass_utils import run_bass_kernel_spmd
from concourse.masks import make_identity
from concourse.tile_scheduler import N_PROCS
from concourse.vector_clock import ScopedClock, VectorClock

B, C, Q, E = 32, 1024, 128, 512
N_CORES = 8
B_LOC = B // N_CORES  # 4 batches per core
F32 = mybir.dt.float32
# Matmul compute dtype: float32r streams 1 col/cycle (vs 4 for float32)
# at N>=512; data stays fp32 in memory, PE multiplies at reduced precision.
MM_DT = mybir.dt.float32r
TR_DT = mybir.dt.float32r  # pass-through data; f32r streams 1.5 vs 2 cyc/row


class _TileContextSplitDrain(tile.TileContext):
    """The stock tail drain carries one sem-wait per outstanding processor
    on a single Drain instruction; this walrus build rejects >2 waits per
    TPB_CTRL instruction. Split them across single-wait NOPs on SP (program
    order on the SP sequencer makes the following drain safe)."""

    def _drain_and_barrier(self, tick_clock, wait_clock):
        nc = self.nc
        gc = tick_clock.global_clock
        for p in range(N_PROCS):
            t = gc[p]
            if t > 0:
                vc = VectorClock([t if i == p else 0 for i in range(N_PROCS)])
                n = nc.sync.nop()
                wait_clock.add_sem_waits(n.ins, ScopedClock({None: vc}))
        nc.sync.drain()
        nc.all_engine_barrier()
        assert self.sems is not None
        popped = nc._tile_sem_poison_stack.pop()
        assert popped is self._sem_poison
        nc.clear_and_free_semaphores(list(self.sems.allocated().values()))
        nc.all_engine_barrier()


def _split_excess_waits(nc, max_waits=1):
    """This walrus build rejects instructions carrying multiple sem-wait
    commands (the exact budget differs per engine codegen; 1 is always safe).
    Move excess waits onto fresh same-engine NOPs inserted directly before
    the instruction (engine program order makes this equivalent)."""
    import bass_rust

    ctr = 0
    for bb in nc.main_func.blocks:
        insts = bb.instructions  # live list backed by the rust block
        i = 0
        while i < len(insts):
            ins = insts[i]
            si = ins.sync_info
            w = list(si.on_wait) if (si and si.on_wait) else []
            if len(w) > max_waits:
                extra, keep = w[:-max_waits], w[-max_waits:]
                for c0 in range(0, len(extra), max_waits):
                    nop = bass_rust.InstNoOp(
                        name=f"wsplit-{ctr}", ins=[], outs=[]
                    )
                    ctr += 1
                    nop.engine = ins.engine
                    nop.sync_info = bass_rust.SyncInfo(
                        on_wait=extra[c0 : c0 + max_waits], on_update=[]
                    )
                    insts.insert(i, nop)
                    i += 1
                ins.sync_info = bass_rust.SyncInfo(
                    on_wait=keep,
                    on_update=list(si.on_update) if si.on_update else [],
                )
            i += 1


def _mm(ap):
    return ap.bitcast(MM_DT)


def _tr(ap):
    return ap.bitcast(TR_DT)


def build_nc() -> bass.Bass:
    nc = bass.Bass(target_bir_lowering=False)
    ctx_d = nc.declare_dram_parameter("context", [B_LOC, C, E], F32, isOutput=False)
    q_d = nc.declare_dram_parameter("question", [B_LOC, Q, E], F32, isOutput=False)
    ws_d = nc.declare_dram_parameter("ws", [3 * E], F32, isOutput=False)
    c2q_d = nc.declare_dram_parameter("c2q", [B_LOC, C, E], F32, isOutput=True)
    q2c_d = nc.declare_dram_parameter("q2c", [B_LOC, 1, E], F32, isOutput=True)

    AX = mybir.AxisListType
    ACTF = mybir.ActivationFunctionType
    NCJ = C // 128  # 8 c-chunks of 128
    NK = E // 128  # 4 e-chunks of 128

    with _TileContextSplitDrain(nc) as tc:
        with (
            tc.tile_pool(name="singles", bufs=1) as singles,
            tc.tile_pool(name="qp", bufs=2) as qp,
            tc.tile_pool(name="cp", bufs=2) as cp,
            tc.tile_pool(name="ctp", bufs=2) as ctp,
            tc.tile_pool(name="bp", bufs=2) as bp,
            tc.tile_pool(name="op", bufs=2) as op,
            tc.tile_pool(name="ps_tp", bufs=5, space="PSUM") as ps_tp,
            tc.tile_pool(name="ps_mm", bufs=2, space="PSUM") as ps_mm,
            tc.tile_pool(name="ps_sm", bufs=1, space="PSUM") as ps_sm,
        ):
            # --- constants: identity, ones, ws rows broadcast on-chip via PE ---
            ident = singles.tile([128, 128], F32)
            make_identity(nc, ident[:, :])
            ones_scr = singles.tile([128, 128], F32)
            nc.vector.memset(ones_scr, 1.0)
            ones_col = singles.tile([128, 1], F32)
            nc.vector.tensor_copy(_mm(ones_col), ones_scr[:, 0:1])
            ones_row = singles.tile([1, 128], F32)
            nc.vector.tensor_copy(_mm(ones_row), ones_scr[0:1, :])
            identr = singles.tile([128, 128], F32)
            nc.vector.tensor_copy(_tr(identr), ident[:, :])
            # ws as a single [1, 1536] row (6 KB DMA), replicated across
            # partitions by a K=1 ones matmul -- keeps 768 KB of broadcast
            # traffic off the DMA critical path and warms the PE early.
            ws_row = singles.tile([1, 3 * E], F32)
            nc.sync.dma_start(
                out=_mm(ws_row),
                in_=_mm(bass.AP(tensor=ws_d, offset=0, ap=[[0, 1], [1, 3 * E]])),
            )
            wq_b = singles.tile([128, E], F32)
            wc_b = singles.tile([128, E], F32)
            wm_b = singles.tile([128, E], F32)
            for wi, wtile in enumerate((wq_b, wc_b, wm_b)):
                wb_ps = ps_mm.tile([128, 512], F32, tag="mm", name=f"wb_ps{wi}")
                nc.tensor.matmul(
                    wb_ps,
                    _mm(ones_row),
                    _mm(ws_row[0:1, wi * E : (wi + 1) * E]),
                    start=True,
                    stop=True,
                )
                nc.scalar.copy(wtile, wb_ps)

            for b in range(B_LOC):
                # ---- question side ----
                q_nat = qp.tile([128, E], F32)
                nc.sync.dma_start(out=_mm(q_nat), in_=_mm(q_d[b, :, :]))
                qs = qp.tile([128, E], F32)
                nc.vector.tensor_mul(qs, q_nat, wm_b)
                qs2 = qp.tile([128, E], F32)
                nc.vector.tensor_add(_tr(qs2), qs, wc_b)
                qd_scr = qp.tile([128, E], F32)
                qdot = qp.tile([128, 1], F32)
                nc.vector.tensor_mul(qd_scr, q_nat, wq_b)
                nc.vector.reduce_sum(qdot, qd_scr, axis=AX.X)
                qsT_ps = ps_tp.tile([128, 512], F32, tag="tp")
                for k in range(NK):
                    nc.tensor.transpose(
                        _tr(qsT_ps[:, 128 * k : 128 * (k + 1)]),
                        _tr(qs2[:, 128 * k : 128 * (k + 1)]),
                        _tr(identr[:, :]),
                    )
                qsT = qp.tile([128, 512], F32)
                nc.scalar.copy(_mm(qsT), qsT_ps)

                # ---- per c-half pipeline: load -> transpose -> sim -> exp
                # -> stats -> c2q -> store (halves are independent except for
                # the shared question side and the final q2c)
                ctx_nat = [None, None]
                maxexp = bp.tile([128, NCJ], F32)
                for h in range(2):
                    cn = cp.tile([128, 4, E], F32, name=f"ctx_nat{h}", tag=f"ctx{h}")
                    ctx_nat[h] = cn  # [p, jj, e]; c = 512h + 128jj + p
                    nc.sync.dma_start(
                        out=_mm(cn),
                        in_=_mm(
                            ctx_d[b, 512 * h : 512 * (h + 1), :].rearrange(
                                "(j p) e -> p j e", p=128
                            )
                        ),
                    )
                    ctxT = ctp.tile(
                        [128, NK, 512], F32, name=f"ctxT{h}", tag=f"ctxT{h}"
                    )
                    tp_ps = [
                        ps_tp.tile([128, 512], F32, tag="tp", name=f"tp_ps{k}")
                        for k in range(NK)
                    ]
                    for jj in range(4):
                        for k in range(NK):
                            nc.tensor.transpose(
                                _tr(tp_ps[k][:, 128 * jj : 128 * (jj + 1)]),
                                _tr(cn[:, jj, 128 * k : 128 * (k + 1)]),
                                _tr(identr[:, :]),
                            )
                    for k in range(NK):
                        dst = _mm(ctxT[:, k, :])
                        if k % 2 == 0:
                            nc.vector.tensor_copy(dst, tp_ps[k])
                        else:
                            nc.scalar.copy(dst, tp_ps[k])
                    # simT[q, c-half] = qs2T.T @ ctxT   (ws_c folded into qs2)
                    simT_ps = ps_mm.tile([128, 512], F32, tag="mm")
                    for k in range(NK):
                        nc.tensor.matmul(
                            simT_ps,
                            _mm(qsT[:, 128 * k : 128 * (k + 1)]),
                            _mm(ctxT[:, k, :]),
                            start=(k == 0),
                            stop=(k == NK - 1),
                        )
                    et = bp.tile([128, 512], F32, tag=f"expsimT{h}", name=f"et{h}")
                    nc.scalar.activation(
                        _mm(et), simT_ps, ACTF.Exp, bias=qdot, scale=1.0
                    )

                    # softmax stats for this half via transposed exp(sim)
                    st_ps = ps_tp.tile([128, 512], F32, tag="tp")
                    for j2 in range(4):
                        nc.tensor.transpose(
                            _tr(st_ps[:, 128 * j2 : 128 * (j2 + 1)]),
                            _tr(et[:, 128 * j2 : 128 * (j2 + 1)]),
                            _tr(identr[:, :]),
                        )
                    v3 = st_ps.rearrange("p (j q) -> p j q", q=128)
                    nc.vector.reduce_max(
                        _mm(maxexp[:, 4 * h : 4 * h + 4]), v3, axis=AX.X
                    )
                    ssum = bp.tile([128, 4], F32, name=f"ssum{h}", tag=f"ssum{h}")
                    nc.vector.reduce_sum(ssum, v3, axis=AX.X)
                    recip = bp.tile([128, 4], F32, name=f"recip{h}", tag=f"recip{h}")
                    nc.vector.reciprocal(recip, ssum)

                    # c2q for this half
                    c2q_st = op.tile(
                        [128, 4, E], F32, name=f"c2q_st{h}", tag=f"c2q{h}"
                    )
                    for jj in range(4):
                        c2q_ps = ps_mm.tile([128, 512], F32, tag="mm")
                        nc.tensor.matmul(
                            c2q_ps,
                            _mm(et[:, 128 * jj : 128 * (jj + 1)]),
                            _mm(q_nat),
                            start=True,
                            stop=True,
                        )
                        if jj % 2 == 0:
                            nc.scalar.activation(
                                c2q_st[:, jj, :],
                                c2q_ps,
                                ACTF.Copy,
                                bias=0.0,
                                scale=recip[:, jj : jj + 1],
                            )
                        else:
                            nc.vector.tensor_scalar_mul(
                                c2q_st[:, jj, :], c2q_ps, recip[:, jj : jj + 1]
                            )
                    nc.sync.dma_start(
                        out=c2q_d[b, 512 * h : 512 * (h + 1), :].rearrange(
                            "(j p) e -> p j e", p=128
                        ),
                        in_=c2q_st,
                    )

                # ---- q2c = softmax_c(max_q sim) @ context ----
                tot_ps = ps_sm.tile([1, 512], F32, tag="sm")
                nc.tensor.matmul(
                    tot_ps[0:1, 0:NCJ],
                    _mm(ones_col),
                    _mm(maxexp),
                    start=True,
                    stop=True,
                )
                tot_sb = bp.tile([1, NCJ], F32)
                nc.scalar.copy(tot_sb, tot_ps[0:1, 0:NCJ])
                tot1 = bp.tile([1, 1], F32)
                nc.vector.reduce_sum(tot1, tot_sb, axis=AX.X)
                rtot = bp.tile([1, 1], F32)
                nc.vector.reciprocal(rtot, tot1)
                q2c_ps = ps_sm.tile([1, 512], F32, tag="sm")
                for j in range(NCJ):
                    h, jj = divmod(j, 4)
                    nc.tensor.matmul(
                        q2c_ps,
                        _mm(maxexp[:, j : j + 1]),
                        _mm(ctx_nat[h][:, jj, :]),
                        start=(j == 0),
                        stop=(j == NCJ - 1),
                    )
                q2c_sb = op.tile([1, E], F32)
                nc.scalar.activation(
                    q2c_sb, q2c_ps, ACTF.Copy, bias=0.0, scale=rtot[0:1, 0:1]
                )
                nc.sync.dma_start(out=q2c_d[b, 0:1, :], in_=q2c_sb)

    _split_excess_waits(nc)
    return nc


_NC_CACHE = None


def _get_nc():
    global _NC_CACHE
    if _NC_CACHE is None:
        _NC_CACHE = build_nc()
    return _NC_CACHE


def kernel(context: np.ndarray, question: np.ndarray, ws: np.ndarray):
    context = np.ascontiguousarray(context, dtype=np.float32)
    question = np.ascontiguousarray(question, dtype=np.float32)
    ws = np.ascontiguousarray(ws, dtype=np.float32)
    nc = _get_nc()
    in_maps = []
    for i in range(N_CORES):
        sl = slice(i * B_LOC, (i + 1) * B_LOC)
        in_maps.append(
            {"context": context[sl], "question": question[sl], "ws": ws}
        )
    res = run_bass_kernel_spmd(nc, in_maps, list(range(N_CORES)))
    c2q = np.concatenate([res.results[i]["c2q"] for i in range(N_CORES)], axis=0)
    q2c = np.concatenate([res.results[i]["q2c"] for i in range(N_CORES)], axis=0)
    return c2q, q2c


if __name__ == "__main__":
    ctx = np.random.randn(B, C, E).astype(np.float32)
    qn = np.random.randn(B, Q, E).astype(np.float32)
    w = (np.random.randn(3 * E) / np.sqrt(E)).astype(np.float32)
    out = kernel(ctx, qn, w)
    print("c2q", out[0].shape, "q2c", out[1].shape)


# revision 20
# speedup vs baseline: 1.0307x; 1.0307x over previous
"""Bass/Trainium2 kernel for BiDAF-style bidirectional attention.

Problem shapes (hardcoded per the task contract):
  context  (32, 1024, 512) f32
  question (32,  128, 512) f32
  ws       (1536,)         f32   = [ws_q | ws_c | ws_m]
Returns (c2q (32,1024,512), q2c (32,1,512)) matching reference.py.

Sharding: data-parallel over batch. 8 cores x 4 batches each; one SPMD
Bass module processes 4 batches per core.

Per-core per-batch dataflow (C=1024, Q=128, E=512):
  sim[c,q] = cdot[c] + qdot[q] + sum_e ctx[c,e]*ws_m[e]*q[q,e]
  - load ctx natural [c,e]; PE-transpose into ctxT [e,c] (4 e-chunks x 1024)
  - qs = q * ws_m (bcast); qsT via PE transpose; qdot via DVE fused
    mul+reduce; cdot row via 8 matmuls with 1-col ws_c weights over ctxT
  - simT[q,c] = qsT.T @ ctxT (+ cdot via an extra K=1 accumulating matmul)
  - expsimT = exp(simT + qdot) fused in the PSUM->SBUF ACT evacuation
    (no max subtraction: |sim| <~ 15 for this data, exp stays in fp32 range;
     softmax is shift-invariant so the result matches the reference)
  - PE-transpose expsimT back to [c,q]; DVE reduce gives s[c] = sum_q exp
    and w[c] = max_q exp = exp(max_q sim)  (the q2c softmax numerator!)
  - c2q[c,:] = (expsimT cols).T @ question, scaled by 1/s[c] during the
    ACT Copy evacuation (per-partition scale)
  - q2c = sum_c w[c]*ctx[c,:] / sum_c w[c] via 1-col-weight matmuls
"""

import os
import sys

for _p in ("/opt/trn_rl_repo",):
    if _p not in sys.path:
        sys.path.insert(0, _p)

import numpy as np

import concourse.bass as bass
import concourse.tile as tile
from concourse import mybir
from concourse.bass_utils import run_bass_kernel_spmd
from concourse.masks import make_identity
from concourse.tile_scheduler import N_PROCS
from concourse.vector_clock import ScopedClock, VectorClock

B, C, Q, E = 32, 1024, 128, 512
N_CORES = 8
B_LOC = B // N_CORES  # 4 batches per core
F32 = mybir.dt.float32
# Matmul compute dtype: float32r streams 1 col/cycle (vs 4 for float32)
# at N>=512; data stays fp32 in memory, PE multiplies at reduced precision.
MM_DT = mybir.dt.float32r
TR_DT = mybir.dt.float32r  # pass-through data; f32r streams 1.5 vs 2 cyc/row


class _TileContextSplitDrain(tile.TileContext):
    """The stock tail drain carries one sem-wait per outstanding processor
    on a single Drain instruction; this walrus build rejects >2 waits per
    TPB_CTRL instruction. Split them across single-wait NOPs on SP (program
    order on the SP sequencer makes the following drain safe)."""

    def _drain_and_barrier(self, tick_clock, wait_clock):
        nc = self.nc
        gc = tick_clock.global_clock
        for p in range(N_PROCS):
            t = gc[p]
            if t > 0:
                vc = VectorClock([t if i == p else 0 for i in range(N_PROCS)])
                n = nc.sync.nop()
                wait_clock.add_sem_waits(n.ins, ScopedClock({None: vc}))
        nc.sync.drain()
        nc.all_engine_barrier()
        assert self.sems is not None
        popped = nc._tile_sem_poison_stack.pop()
        assert popped is self._sem_poison
        nc.clear_and_free_semaphores(list(self.sems.allocated().values()))
        nc.all_engine_barrier()


def _split_excess_waits(nc, max_waits=1):
    """This walrus build rejects instructions carrying multiple sem-wait
    commands (the exact budget differs per engine codegen; 1 is always safe).
    Move excess waits onto fresh same-engine NOPs inserted directly before
    the instruction (engine program order makes this equivalent)."""
    import bass_rust

    ctr = 0
    for bb in nc.main_func.blocks:
        insts = bb.instructions  # live list backed by the rust block
        i = 0
        while i < len(insts):
            ins = insts[i]
            si = ins.sync_info
            w = list(si.on_wait) if (si and si.on_wait) else []
            if len(w) > max_waits:
                extra, keep = w[:-max_waits], w[-max_waits:]
                for c0 in range(0, len(extra), max_waits):
                    nop = bass_rust.InstNoOp(
                        name=f"wsplit-{ctr}", ins=[], outs=[]
                    )
                    ctr += 1
                    nop.engine = ins.engine
                    nop.sync_info = bass_rust.SyncInfo(
                        on_wait=extra[c0 : c0 + max_waits], on_update=[]
                    )
                    insts.insert(i, nop)
                    i += 1
                ins.sync_info = bass_rust.SyncInfo(
                    on_wait=keep,
                    on_update=list(si.on_update) if si.on_update else [],
                )
            i += 1


def _mm(ap):
    return ap.bitcast(MM_DT)


def _tr(ap):
    return ap.bitcast(TR_DT)


def build_nc() -> bass.Bass:
    nc = bass.Bass(target_bir_lowering=False)
    ctx_d = nc.declare_dram_parameter("context", [B_LOC, C, E], F32, isOutput=False)
    q_d = nc.declare_dram_parameter("question", [B_LOC, Q, E], F32, isOutput=False)
    ws_d = nc.declare_dram_parameter("ws", [3 * E], F32, isOutput=False)
    c2q_d = nc.declare_dram_parameter("c2q", [B_LOC, C, E], F32, isOutput=True)
    q2c_d = nc.declare_dram_parameter("q2c", [B_LOC, 1, E], F32, isOutput=True)

    AX = mybir.AxisListType
    ACTF = mybir.ActivationFunctionType
    NCJ = C // 128  # 8 c-chunks of 128
    NK = E // 128  # 4 e-chunks of 128

    with _TileContextSplitDrain(nc) as tc:
        with (
            tc.tile_pool(name="singles", bufs=1) as singles,
            tc.tile_pool(name="qp", bufs=2) as qp,
            tc.tile_pool(name="cp", bufs=2) as cp,
            tc.tile_pool(name="ctp", bufs=2) as ctp,
            tc.tile_pool(name="bp", bufs=2) as bp,
            tc.tile_pool(name="op", bufs=2) as op,
            tc.tile_pool(name="ps_tp", bufs=5, space="PSUM") as ps_tp,
            tc.tile_pool(name="ps_mm", bufs=2, space="PSUM") as ps_mm,
            tc.tile_pool(name="ps_sm", bufs=1, space="PSUM") as ps_sm,
        ):
            # --- constants: identity, ones, ws rows broadcast on-chip via PE ---
            ident = singles.tile([128, 128], F32)
            make_identity(nc, ident[:, :])
            ones_scr = singles.tile([128, 128], F32)
            nc.vector.memset(ones_scr, 1.0)
            ones_col = singles.tile([128, 1], F32)
            nc.vector.tensor_copy(_mm(ones_col), ones_scr[:, 0:1])
            ones_row = singles.tile([1, 128], F32)
            nc.vector.tensor_copy(_mm(ones_row), ones_scr[0:1, :])
            identr = singles.tile([128, 128], F32)
            nc.vector.tensor_copy(_tr(identr), ident[:, :])
            # ws as a single [1, 1536] row (6 KB DMA), replicated across
            # partitions by a K=1 ones matmul -- keeps 768 KB of broadcast
            # traffic off the DMA critical path and warms the PE early.
            ws_row = singles.tile([1, 3 * E], F32)
            nc.sync.dma_start(
                out=_mm(ws_row),
                in_=_mm(bass.AP(tensor=ws_d, offset=0, ap=[[0, 1], [1, 3 * E]])),
            )
            wq_b = singles.tile([128, E], F32)
            wc_b = singles.tile([128, E], F32)
            wm_b = singles.tile([128, E], F32)
            for wi, wtile in enumerate((wq_b, wc_b, wm_b)):
                wb_ps = ps_mm.tile([128, 512], F32, tag="mm", name=f"wb_ps{wi}")
                nc.tensor.matmul(
                    wb_ps,
                    _mm(ones_row),
                    _mm(ws_row[0:1, wi * E : (wi + 1) * E]),
                    start=True,
                    stop=True,
                )
                nc.scalar.copy(wtile, wb_ps)

            for b in range(B_LOC):
                # ---- question side ----
                q_nat = qp.tile([128, E], F32)
                nc.sync.dma_start(out=_mm(q_nat), in_=_mm(q_d[b, :, :]))
                qs = qp.tile([128, E], F32)
                nc.vector.tensor_mul(qs, q_nat, wm_b)
                qs2 = qp.tile([128, E], F32)
                nc.vector.tensor_add(_tr(qs2), qs, wc_b)
                qd_scr = qp.tile([128, E], F32)
                qdot = qp.tile([128, 1], F32)
                nc.vector.tensor_mul(qd_scr, q_nat, wq_b)
                nc.vector.reduce_sum(qdot, qd_scr, axis=AX.X)
                qsT_ps = ps_tp.tile([128, 512], F32, tag="tp")
                for k in range(NK):
                    nc.tensor.transpose(
                        _tr(qsT_ps[:, 128 * k : 128 * (k + 1)]),
                        _tr(qs2[:, 128 * k : 128 * (k + 1)]),
                        _tr(identr[:, :]),
                    )
                qsT = qp.tile([128, 512], F32)
                nc.scalar.copy(_mm(qsT), qsT_ps)

                # ---- per c-half pipeline: load -> transpose -> sim -> exp
                # -> stats -> c2q -> store (halves are independent except for
                # the shared question side and the final q2c)
                ctx_nat = [None, None]
                expsimT = [None, None]
                maxexp = bp.tile([128, NCJ], F32)
                for h in range(2):
                    cn = cp.tile([128, 4, E], F32, name=f"ctx_nat{h}", tag=f"ctx{h}")
                    ctx_nat[h] = cn  # [p, jj, e]; c = 512h + 128jj + p
                    nc.sync.dma_start(
                        out=_mm(cn),
                        in_=_mm(
                            ctx_d[b, 512 * h : 512 * (h + 1), :].rearrange(
                                "(j p) e -> p j e", p=128
                            )
                        ),
                    )
                    ctxT = ctp.tile(
                        [128, NK, 512], F32, name=f"ctxT{h}", tag=f"ctxT{h}"
                    )
                    tp_ps = [
                        ps_tp.tile([128, 512], F32, tag="tp", name=f"tp_ps{k}")
                        for k in range(NK)
                    ]
                    for jj in range(4):
                        for k in range(NK):
                            nc.tensor.transpose(
                                _tr(tp_ps[k][:, 128 * jj : 128 * (jj + 1)]),
                                _tr(cn[:, jj, 128 * k : 128 * (k + 1)]),
                                _tr(identr[:, :]),
                            )
                    for k in range(NK):
                        dst = _mm(ctxT[:, k, :])
                        if k % 2 == 0:
                            nc.vector.tensor_copy(dst, tp_ps[k])
                        else:
                            nc.scalar.copy(dst, tp_ps[k])
                    # simT[q, c-half] = qs2T.T @ ctxT   (ws_c folded into qs2)
                    simT_ps = ps_mm.tile([128, 512], F32, tag="mm")
                    for k in range(NK):
                        nc.tensor.matmul(
                            simT_ps,
                            _mm(qsT[:, 128 * k : 128 * (k + 1)]),
                            _mm(ctxT[:, k, :]),
                            start=(k == 0),
                            stop=(k == NK - 1),
                        )
                    et = bp.tile([128, 512], F32, tag=f"expsimT{h}", name=f"et{h}")
                    expsimT[h] = et
                    nc.scalar.activation(
                        _mm(et), simT_ps, ACTF.Exp, bias=qdot, scale=1.0
                    )

                # ---- softmax stats per half via transposed exp(sim) ----
                recips = [None, None]
                for h in range(2):
                    st_ps = ps_tp.tile([128, 512], F32, tag="tp")
                    for j2 in range(4):
                        nc.tensor.transpose(
                            _tr(st_ps[:, 128 * j2 : 128 * (j2 + 1)]),
                            _tr(expsimT[h][:, 128 * j2 : 128 * (j2 + 1)]),
                            _tr(identr[:, :]),
                        )
                    v3 = st_ps.rearrange("p (j q) -> p j q", q=128)
                    nc.vector.reduce_max(
                        _mm(maxexp[:, 4 * h : 4 * h + 4]), v3, axis=AX.X
                    )
                    ssum = bp.tile([128, 4], F32, name=f"ssum{h}", tag=f"ssum{h}")
                    nc.vector.reduce_sum(ssum, v3, axis=AX.X)
                    recip = bp.tile([128, 4], F32, name=f"recip{h}", tag=f"recip{h}")
                    nc.vector.reciprocal(recip, ssum)
                    recips[h] = recip

                # ---- c2q = softmax_q(sim) @ question (per-half store) ----
                for h in range(2):
                    c2q_st = op.tile(
                        [128, 4, E], F32, name=f"c2q_st{h}", tag=f"c2q{h}"
                    )
                    for jj in range(4):
                        c2q_ps = ps_mm.tile([128, 512], F32, tag="mm")
                        nc.tensor.matmul(
                            c2q_ps,
                            _mm(expsimT[h][:, 128 * jj : 128 * (jj + 1)]),
                            _mm(q_nat),
                            start=True,
                            stop=True,
                        )
                        nc.scalar.activation(
                            c2q_st[:, jj, :],
                            c2q_ps,
                            ACTF.Copy,
                            bias=0.0,
                            scale=recips[h][:, jj : jj + 1],
                        )
                    nc.sync.dma_start(
                        out=c2q_d[b, 512 * h : 512 * (h + 1), :].rearrange(
                            "(j p) e -> p j e", p=128
                        ),
                        in_=c2q_st,
                    )

                # ---- q2c = softmax_c(max_q sim) @ context ----
                tot_ps = ps_sm.tile([1, 512], F32, tag="sm")
                nc.tensor.matmul(
                    tot_ps[0:1, 0:NCJ],
                    _mm(ones_col),
                    _mm(maxexp),
                    start=True,
                    stop=True,
                )
                tot_sb = bp.tile([1, NCJ], F32)
                nc.scalar.copy(tot_sb, tot_ps[0:1, 0:NCJ])
                tot1 = bp.tile([1, 1], F32)
                nc.vector.reduce_sum(tot1, tot_sb, axis=AX.X)
                rtot = bp.tile([1, 1], F32)
                nc.vector.reciprocal(rtot, tot1)
                q2c_ps = ps_sm.tile([1, 512], F32, tag="sm")
                for j in range(NCJ):
                    h, jj = divmod(j, 4)
                    nc.tensor.matmul(
                        q2c_ps,
                        _mm(maxexp[:, j : j + 1]),
                        _mm(ctx_nat[h][:, jj, :]),
                        start=(j == 0),
                        stop=(j == NCJ - 1),
                    )
                q2c_sb = op.tile([1, E], F32)
                nc.scalar.activation(
                    q2c_sb, q2c_ps, ACTF.Copy, bias=0.0, scale=rtot[0:1, 0:1]
                )
                nc.sync.dma_start(out=q2c_d[b, 0:1, :], in_=q2c_sb)

    _split_excess_waits(nc)
    return nc


_NC_CACHE = None


def _get_nc():
    global _NC_CACHE
    if _NC_CACHE is None:
        _NC_CACHE = build_nc()
    return _NC_CACHE


def kernel(context: np.ndarray, question: np.ndarray, ws: np.ndarray):
    context = np.ascontiguousarray(context, dtype=np.float32)
    question = np.ascontiguousarray(question, dtype=np.float32)
    ws = np.ascontiguousarray(ws, dtype=np.float32)
    nc = _get_nc()
    in_maps = []
    for i in range(N_CORES):
        sl = slice(i * B_LOC, (i + 1) * B_LOC)
        in_maps.append(
            {"context": context[sl], "question": question[sl], "ws": ws}
        )
    res = run_bass_kernel_spmd(nc, in_maps, list(range(N_CORES)))
    c2q = np.concatenate([res.results[i]["c2q"] for i in range(N_CORES)], axis=0)
    q2c = np.concatenate([res.results[i]["q2c"] for i in range(N_CORES)], axis=0)
    return c2q, q2c


if __name__ == "__main__":
    ctx = np.random.randn(B, C, E).astype(np.float32)
    qn = np.random.randn(B, Q, E).astype(np.float32)
    w = (np.random.randn(3 * E) / np.sqrt(E)).astype(np.float32)
    out = kernel(ctx, qn, w)
    print("c2q", out[0].shape, "q2c", out[1].shape)


# revision 21
# speedup vs baseline: 1.2704x; 1.2326x over previous
"""Bass/Trainium2 kernel for BiDAF-style bidirectional attention.

Problem shapes (hardcoded per the task contract):
  context  (32, 1024, 512) f32
  question (32,  128, 512) f32
  ws       (1536,)         f32   = [ws_q | ws_c | ws_m]
Returns (c2q (32,1024,512), q2c (32,1,512)) matching reference.py.

Sharding: data-parallel over batch. 8 cores x 4 batches each; one SPMD
Bass module processes 4 batches per core.

Per-core per-batch dataflow (C=1024, Q=128, E=512):
  sim[c,q] = cdot[c] + qdot[q] + sum_e ctx[c,e]*ws_m[e]*q[q,e]
  - load ctx natural [c,e]; PE-transpose into ctxT [e,c] (4 e-chunks x 1024)
  - qs = q * ws_m (bcast); qsT via PE transpose; qdot via DVE fused
    mul+reduce; cdot row via 8 matmuls with 1-col ws_c weights over ctxT
  - simT[q,c] = qsT.T @ ctxT (+ cdot via an extra K=1 accumulating matmul)
  - expsimT = exp(simT + qdot) fused in the PSUM->SBUF ACT evacuation
    (no max subtraction: |sim| <~ 15 for this data, exp stays in fp32 range;
     softmax is shift-invariant so the result matches the reference)
  - PE-transpose expsimT back to [c,q]; DVE reduce gives s[c] = sum_q exp
    and w[c] = max_q exp = exp(max_q sim)  (the q2c softmax numerator!)
  - c2q[c,:] = (expsimT cols).T @ question, scaled by 1/s[c] during the
    ACT Copy evacuation (per-partition scale)
  - q2c = sum_c w[c]*ctx[c,:] / sum_c w[c] via 1-col-weight matmuls
"""

import os
import sys

for _p in ("/opt/trn_rl_repo",):
    if _p not in sys.path:
        sys.path.insert(0, _p)

import numpy as np

import concourse.bass as bass
import concourse.tile as tile
from concourse import mybir
from concourse.bass_utils import run_bass_kernel_spmd
from concourse.masks import make_identity
from concourse.tile_scheduler import N_PROCS
from concourse.vector_clock import ScopedClock, VectorClock

B, C, Q, E = 32, 1024, 128, 512
N_CORES = 8
B_LOC = B // N_CORES  # 4 batches per core
F32 = mybir.dt.float32
# Matmul compute dtype: float32r streams 1 col/cycle (vs 4 for float32)
# at N>=512; data stays fp32 in memory, PE multiplies at reduced precision.
MM_DT = mybir.dt.float32r
TR_DT = mybir.dt.float32r  # pass-through data; f32r streams 1.5 vs 2 cyc/row


class _TileContextSplitDrain(tile.TileContext):
    """The stock tail drain carries one sem-wait per outstanding processor
    on a single Drain instruction; this walrus build rejects >2 waits per
    TPB_CTRL instruction. Split them across single-wait NOPs on SP (program
    order on the SP sequencer makes the following drain safe)."""

    def _drain_and_barrier(self, tick_clock, wait_clock):
        nc = self.nc
        gc = tick_clock.global_clock
        for p in range(N_PROCS):
            t = gc[p]
            if t > 0:
                vc = VectorClock([t if i == p else 0 for i in range(N_PROCS)])
                n = nc.sync.nop()
                wait_clock.add_sem_waits(n.ins, ScopedClock({None: vc}))
        nc.sync.drain()
        nc.all_engine_barrier()
        assert self.sems is not None
        popped = nc._tile_sem_poison_stack.pop()
        assert popped is self._sem_poison
        nc.clear_and_free_semaphores(list(self.sems.allocated().values()))
        nc.all_engine_barrier()


def _split_excess_waits(nc, max_waits=1):
    """This walrus build rejects instructions carrying multiple sem-wait
    commands (the exact budget differs per engine codegen; 1 is always safe).
    Move excess waits onto fresh same-engine NOPs inserted directly before
    the instruction (engine program order makes this equivalent)."""
    import bass_rust

    ctr = 0
    for bb in nc.main_func.blocks:
        insts = bb.instructions  # live list backed by the rust block
        i = 0
        while i < len(insts):
            ins = insts[i]
            si = ins.sync_info
            w = list(si.on_wait) if (si and si.on_wait) else []
            if len(w) > max_waits:
                extra, keep = w[:-max_waits], w[-max_waits:]
                for c0 in range(0, len(extra), max_waits):
                    nop = bass_rust.InstNoOp(
                        name=f"wsplit-{ctr}", ins=[], outs=[]
                    )
                    ctr += 1
                    nop.engine = ins.engine
                    nop.sync_info = bass_rust.SyncInfo(
                        on_wait=extra[c0 : c0 + max_waits], on_update=[]
                    )
                    insts.insert(i, nop)
                    i += 1
                ins.sync_info = bass_rust.SyncInfo(
                    on_wait=keep,
                    on_update=list(si.on_update) if si.on_update else [],
                )
            i += 1


def _mm(ap):
    return ap.bitcast(MM_DT)


def _tr(ap):
    return ap.bitcast(TR_DT)


def build_nc() -> bass.Bass:
    nc = bass.Bass(target_bir_lowering=False)
    ctx_d = nc.declare_dram_parameter("context", [B_LOC, C, E], F32, isOutput=False)
    q_d = nc.declare_dram_parameter("question", [B_LOC, Q, E], F32, isOutput=False)
    ws_d = nc.declare_dram_parameter("ws", [3 * E], F32, isOutput=False)
    c2q_d = nc.declare_dram_parameter("c2q", [B_LOC, C, E], F32, isOutput=True)
    q2c_d = nc.declare_dram_parameter("q2c", [B_LOC, 1, E], F32, isOutput=True)

    AX = mybir.AxisListType
    ACTF = mybir.ActivationFunctionType
    NCJ = C // 128  # 8 c-chunks of 128
    NK = E // 128  # 4 e-chunks of 128

    with _TileContextSplitDrain(nc) as tc:
        with (
            tc.tile_pool(name="singles", bufs=1) as singles,
            tc.tile_pool(name="qp", bufs=3) as qp,
            tc.tile_pool(name="cp", bufs=3) as cp,
            tc.tile_pool(name="ctp", bufs=2) as ctp,
            tc.tile_pool(name="bp", bufs=2) as bp,
            tc.tile_pool(name="op", bufs=2) as op,
            tc.tile_pool(name="ps_tp", bufs=5, space="PSUM") as ps_tp,
            tc.tile_pool(name="ps_mm", bufs=2, space="PSUM") as ps_mm,
            tc.tile_pool(name="ps_sm", bufs=1, space="PSUM") as ps_sm,
        ):
            # --- constants: identity, ones, ws rows broadcast on-chip via PE ---
            ident = singles.tile([128, 128], F32)
            make_identity(nc, ident[:, :])
            ones_scr = singles.tile([128, 128], F32)
            nc.vector.memset(ones_scr, 1.0)
            ones_col = singles.tile([128, 1], F32)
            nc.vector.tensor_copy(_mm(ones_col), ones_scr[:, 0:1])
            ones_row = singles.tile([1, 128], F32)
            nc.vector.tensor_copy(_mm(ones_row), ones_scr[0:1, :])
            identr = singles.tile([128, 128], F32)
            nc.vector.tensor_copy(_tr(identr), ident[:, :])
            # ws as a single [1, 1536] row (6 KB DMA), replicated across
            # partitions by a K=1 ones matmul -- keeps 768 KB of broadcast
            # traffic off the DMA critical path and warms the PE early.
            ws_row = singles.tile([1, 3 * E], F32)
            nc.sync.dma_start(
                out=_mm(ws_row),
                in_=_mm(bass.AP(tensor=ws_d, offset=0, ap=[[0, 1], [1, 3 * E]])),
            )
            wq_b = singles.tile([128, E], F32)
            wc_b = singles.tile([128, E], F32)
            wm_b = singles.tile([128, E], F32)
            for wi, wtile in enumerate((wq_b, wc_b, wm_b)):
                wb_ps = ps_mm.tile([128, 512], F32, tag="mm", name=f"wb_ps{wi}")
                nc.tensor.matmul(
                    wb_ps,
                    _mm(ones_row),
                    _mm(ws_row[0:1, wi * E : (wi + 1) * E]),
                    start=True,
                    stop=True,
                )
                nc.scalar.copy(wtile, wb_ps)

            for b in range(B_LOC):
                # ---- question side ----
                q_nat = qp.tile([128, E], F32)
                nc.sync.dma_start(out=_mm(q_nat), in_=_mm(q_d[b, :, :]))
                qs = qp.tile([128, E], F32)
                nc.vector.tensor_mul(qs, q_nat, wm_b)
                qs2 = qp.tile([128, E], F32)
                nc.vector.tensor_add(_tr(qs2), qs, wc_b)
                qd_scr = qp.tile([128, E], F32)
                qdot = qp.tile([128, 1], F32)
                nc.vector.tensor_mul(qd_scr, q_nat, wq_b)
                nc.vector.reduce_sum(qdot, qd_scr, axis=AX.X)
                qsT_ps = ps_tp.tile([128, 512], F32, tag="tp")
                for k in range(NK):
                    nc.tensor.transpose(
                        _tr(qsT_ps[:, 128 * k : 128 * (k + 1)]),
                        _tr(qs2[:, 128 * k : 128 * (k + 1)]),
                        _tr(identr[:, :]),
                    )
                qsT = qp.tile([128, 512], F32)
                nc.scalar.copy(_mm(qsT), qsT_ps)

                # ---- per c-half pipeline: load -> transpose -> sim -> exp
                # -> stats -> c2q -> store (halves are independent except for
                # the shared question side and the final q2c)
                ctx_nat = [None, None]
                expsimT = [None, None]
                maxexp = bp.tile([128, NCJ], F32)
                for h in range(2):
                    cn = cp.tile([128, 4, E], F32, name=f"ctx_nat{h}", tag=f"ctx{h}")
                    ctx_nat[h] = cn  # [p, jj, e]; c = 512h + 128jj + p
                    nc.sync.dma_start(
                        out=_mm(cn),
                        in_=_mm(
                            ctx_d[b, 512 * h : 512 * (h + 1), :].rearrange(
                                "(j p) e -> p j e", p=128
                            )
                        ),
                    )
                    ctxT = ctp.tile(
                        [128, NK, 512], F32, name=f"ctxT{h}", tag=f"ctxT{h}"
                    )
                    tp_ps = [
                        ps_tp.tile([128, 512], F32, tag="tp", name=f"tp_ps{k}")
                        for k in range(NK)
                    ]
                    for jj in range(4):
                        for k in range(NK):
                            nc.tensor.transpose(
                                _tr(tp_ps[k][:, 128 * jj : 128 * (jj + 1)]),
                                _tr(cn[:, jj, 128 * k : 128 * (k + 1)]),
                                _tr(identr[:, :]),
                            )
                    for k in range(NK):
                        dst = _mm(ctxT[:, k, :])
                        if k % 2 == 0:
                            nc.vector.tensor_copy(dst, tp_ps[k])
                        else:
                            nc.scalar.copy(dst, tp_ps[k])
                    # simT[q, c-half] = qs2T.T @ ctxT   (ws_c folded into qs2)
                    simT_ps = ps_mm.tile([128, 512], F32, tag="mm")
                    for k in range(NK):
                        nc.tensor.matmul(
                            simT_ps,
                            _mm(qsT[:, 128 * k : 128 * (k + 1)]),
                            _mm(ctxT[:, k, :]),
                            start=(k == 0),
                            stop=(k == NK - 1),
                        )
                    et = bp.tile([128, 512], F32, tag=f"expsimT{h}", name=f"et{h}")
                    expsimT[h] = et
                    nc.scalar.activation(
                        _mm(et), simT_ps, ACTF.Exp, bias=qdot, scale=1.0
                    )

                # ---- softmax stats per half via transposed exp(sim) ----
                recips = [None, None]
                for h in range(2):
                    st_ps = ps_tp.tile([128, 512], F32, tag="tp")
                    for j2 in range(4):
                        nc.tensor.transpose(
                            _tr(st_ps[:, 128 * j2 : 128 * (j2 + 1)]),
                            _tr(expsimT[h][:, 128 * j2 : 128 * (j2 + 1)]),
                            _tr(identr[:, :]),
                        )
                    v3 = st_ps.rearrange("p (j q) -> p j q", q=128)
                    nc.vector.reduce_max(
                        _mm(maxexp[:, 4 * h : 4 * h + 4]), v3, axis=AX.X
                    )
                    ssum = bp.tile([128, 4], F32, name=f"ssum{h}", tag=f"ssum{h}")
                    nc.vector.reduce_sum(ssum, v3, axis=AX.X)
                    recip = bp.tile([128, 4], F32, name=f"recip{h}", tag=f"recip{h}")
                    nc.vector.reciprocal(recip, ssum)
                    recips[h] = recip

                # ---- c2q = softmax_q(sim) @ question (per-half store) ----
                for h in range(2):
                    c2q_st = op.tile(
                        [128, 4, E], F32, name=f"c2q_st{h}", tag=f"c2q{h}"
                    )
                    for jj in range(4):
                        c2q_ps = ps_mm.tile([128, 512], F32, tag="mm")
                        nc.tensor.matmul(
                            c2q_ps,
                            _mm(expsimT[h][:, 128 * jj : 128 * (jj + 1)]),
                            _mm(q_nat),
                            start=True,
                            stop=True,
                        )
                        nc.scalar.activation(
                            c2q_st[:, jj, :],
                            c2q_ps,
                            ACTF.Copy,
                            bias=0.0,
                            scale=recips[h][:, jj : jj + 1],
                        )
                    nc.sync.dma_start(
                        out=c2q_d[b, 512 * h : 512 * (h + 1), :].rearrange(
                            "(j p) e -> p j e", p=128
                        ),
                        in_=c2q_st,
                    )

                # ---- q2c = softmax_c(max_q sim) @ context ----
                tot_ps = ps_sm.tile([1, 512], F32, tag="sm")
                nc.tensor.matmul(
                    tot_ps[0:1, 0:NCJ],
                    _mm(ones_col),
                    _mm(maxexp),
                    start=True,
                    stop=True,
                )
                tot_sb = bp.tile([1, NCJ], F32)
                nc.scalar.copy(tot_sb, tot_ps[0:1, 0:NCJ])
                tot1 = bp.tile([1, 1], F32)
                nc.vector.reduce_sum(tot1, tot_sb, axis=AX.X)
                rtot = bp.tile([1, 1], F32)
                nc.vector.reciprocal(rtot, tot1)
                q2c_ps = ps_sm.tile([1, 512], F32, tag="sm")
                for j in range(NCJ):
                    h, jj = divmod(j, 4)
                    nc.tensor.matmul(
                        q2c_ps,
                        _mm(maxexp[:, j : j + 1]),
                        _mm(ctx_nat[h][:, jj, :]),
                        start=(j == 0),
                        stop=(j == NCJ - 1),
                    )
                q2c_sb = op.tile([1, E], F32)
                nc.scalar.activation(
                    q2c_sb, q2c_ps, ACTF.Copy, bias=0.0, scale=rtot[0:1, 0:1]
                )
                nc.sync.dma_start(out=q2c_d[b, 0:1, :], in_=q2c_sb)

    _split_excess_waits(nc)
    return nc


_NC_CACHE = None


def _get_nc():
    global _NC_CACHE
    if _NC_CACHE is None:
        _NC_CACHE = build_nc()
    return _NC_CACHE


def kernel(context: np.ndarray, question: np.ndarray, ws: np.ndarray):
    context = np.ascontiguousarray(context, dtype=np.float32)
    question = np.ascontiguousarray(question, dtype=np.float32)
    ws = np.ascontiguousarray(ws, dtype=np.float32)
    nc = _get_nc()
    in_maps = []
    for i in range(N_CORES):
        sl = slice(i * B_LOC, (i + 1) * B_LOC)
        in_maps.append(
            {"context": context[sl], "question": question[sl], "ws": ws}
        )
    res = run_bass_kernel_spmd(nc, in_maps, list(range(N_CORES)))
    c2q = np.concatenate([res.results[i]["c2q"] for i in range(N_CORES)], axis=0)
    q2c = np.concatenate([res.results[i]["q2c"] for i in range(N_CORES)], axis=0)
    return c2q, q2c


if __name__ == "__main__":
    ctx = np.random.randn(B, C, E).astype(np.float32)
    qn = np.random.randn(B, Q, E).astype(np.float32)
    w = (np.random.randn(3 * E) / np.sqrt(E)).astype(np.float32)
    out = kernel(ctx, qn, w)
    print("c2q", out[0].shape, "q2c", out[1].shape)
